# revision 14
# baseline (speedup 1.0000x reference)
"""Trainium2 Bass kernel for nn_EquivairantMultiheadAttention (sparse attention).

Contract: kernel(**inputs) takes the FULL unsharded numpy inputs (as produced by
setup_inputs()) and returns the FULL (B, N, COUT) float32 output.

Sharding: 8 cores = data-parallel over batch (2) x sequence-parallel over the
query dim n (4 slices of 512). Each core receives its batch's coset_functions
(transposed, bf16), its query-slice of pairwise_g rows (pre-gathered by
neighborhood on the host as part of input sharding) and nbhd_idx, plus all
weights.

Math notes (equivalences used, all exact):
 - t3 = einsum(K, u) and every other per-(n,h) constant (b_k.Q, b_l.(Q+v))
   is constant across the softmax axis m, hence drops out of softmax.
 - e . (Q+v) = pg . G with G[n,h,:] = W_l[h-block,:]^T (Q+v)[n,h-block].
 - b_in is added after the weighted sum (weights sum to 1); the W_out
   projection runs on the PE per query block (it cannot be folded into V:
   attention weights differ per head while W_out mixes channels across heads).
 - mask is all ones (spec: fill=ones) -> masking is a no-op.

Performance structure:
 - Host ships the MINIMUM bytes per core (~2.5MB): coset (bf16, transposed),
   weights (bf16), the 16-partition wrapped neighbor index list (replicated
   to 128 partitions on device), and the neighborhood-gathered pairwise_g
   rows (bf16, 12B each). No dense pairwise_g and no [P,C] bias broadcasts
   cross the host link; bias rows are outer-product-broadcast on the PE.
 - K and V2 rows are built once (PE matmuls) as ONE combined bf16 row
   [K(256)||V2(256)] = 1KB and gathered with a single dma_gather per chunk.
   The dma_gather descriptor generation (SWDGE ucode on the Pool engine,
   ~8.5ns/row serialized) is the device-side critical path; everything else
   overlaps under it.
 - All pair-stream elementwise work is bf16 with unit innermost stride so the
   DVE 2x_1p mode applies; grouped reductions are add-trees of TensorTensor
   ops (TensorReduce has no fast mode). Products and trees run IN-PLACE in
   the gather buffer (SBUF pressure).
 - exp and the E->[P,MQ,C] head-broadcast run on the idle Activation engine,
   split per m-quarter so their latency hides behind later quarters' DVE work.
 - E stays UNNORMALIZED; out is scaled by 1/Z per head in the f32 epilogue.
 - KV gathers are tiled per m-QUARTER (16 neighbors, 16KB tiles, 5 buffers):
   finer tiles start t1 sooner after the K||V table is built and free slots
   earlier, so the in-order Pool engine can issue the next query block's
   gathers while this block is still reducing.
"""

import math
import sys

import numpy as np

sys.path.insert(0, "/opt/trn_rl_repo")

B, N, M = 2, 2048, 64
C = 256  # CIN == COUT
H, D, POS = 8, 32, 6
NQ = 512  # queries per core
QB = 4  # query blocks of 128 per core
P = 128
NCORES = 8
INV_SQRT_D = 1.0 / math.sqrt(D)
NIC = 1024  # max idxs per dma_gather call (HW: larger fails the exec unit)
MH = M // 2  # m-half

_compiled = {}


def build_bass():
    import concourse.bacc as bacc
    import concourse.mybir as mybir
    import concourse.tile as tile

    dt = mybir.dt
    nc = bacc.Bacc("TRN2", target_bir_lowering=False, debug=False,
                   enable_asserts=False, num_devices=NCORES,
                   num_swdge_queues=4, dynamic_dma_scratch_size=1 << 15)

    f32 = dt.float32
    bf16 = dt.bfloat16
    i16 = dt.int16

    # ---- DRAM inputs (per core) ----
    d_cosetT = nc.dram_tensor("cosetT", [2, P, N], bf16, kind="ExternalInput")
    d_cosetQT = nc.dram_tensor("cosetQT", [2, P, NQ], bf16,
                               kind="ExternalInput")
    d_wqT = nc.dram_tensor("wqT", [2, P, C], bf16, kind="ExternalInput")
    d_wkvT = nc.dram_tensor("wkvT", [2, P, 2 * C], bf16, kind="ExternalInput")
    d_woT = nc.dram_tensor("woT", [2, P, C], bf16, kind="ExternalInput")
    d_wlBD = nc.dram_tensor("wlBD", [2, P, H * POS], bf16,
                            kind="ExternalInput")
    d_bqv = nc.dram_tensor("bqv", [2, P, 1], f32, kind="ExternalInput")
    # bias rows (broadcast to [P, C] on device via ones outer product)
    d_brows = nc.dram_tensor("brows", [1, 3, C], bf16, kind="ExternalInput")
    # wrapped int16 index list (m-major: list pos i = m*128 + n_sub),
    # 16-partition form; replicated to the 8 groups of 16 partitions on device
    d_idxw16 = nc.dram_tensor("idxw16", [16, QB, M * P // 16], i16,
                              kind="ExternalInput")
    # neighborhood-gathered pairwise_g rows, bf16: [n_sub, qb, m, POS]
    d_pgrows = nc.dram_tensor("pgrows", [P, QB, M, POS], bf16,
                              kind="ExternalInput")
    d_out = nc.dram_tensor("out", [QB, P, C], f32, kind="ExternalOutput")

    add = mybir.AluOpType.add
    mult = mybir.AluOpType.mult

    with tile.TileContext(nc) as tc:
        with (
            tc.tile_pool(name="const", bufs=1) as constp,
            tc.tile_pool(name="dram", bufs=1, space="DRAM") as dramp,
            tc.tile_pool(name="evac", bufs=2) as evacp,
        ):
            # critical-path loads first: the qb0 KV gathers gate only on
            # idxw + the kvdr table (cosT + wkvT -> PE build)
            wkvT = constp.tile([P, 2, 2 * C], bf16)
            for cc in range(2):
                nc.sync.dma_start(wkvT[:, cc, :], d_wkvT[cc])

            # combined K||V2 rows in DRAM scratch (bf16, 1KB rows)
            kvdr = dramp.tile([N, 2 * C], bf16)

            q_rows = constp.tile([P, QB, C], bf16)
            g_rows = constp.tile([P, QB, H * POS], bf16)
            binmat = constp.tile([P, C], f32)
            boutmat = constp.tile([P, C], f32)

            with (
                tc.tile_pool(name="prep", bufs=1) as prepp,
                tc.tile_pool(name="psumP", bufs=2, space="PSUM") as psump,
            ):
                # K||V build first: the gathers gate on the full kvdr table.
                # cosT arrives in per-slab column chunks so slab jt's matmul
                # starts as soon as its own 64KB landed, not after the full
                # 1MB load.
                # K||V build first: cosT arrives in per-slab column chunks so
                # slab jt's matmul starts as soon as its own 64KB landed.
                # kvdr writebacks go out on the SECOND HWDGE queue (scalar)
                # so they don't head-of-line block later loads on sync.
                cosT = prepp.tile([P, 2, N], bf16)
                for jt in range(N // P):
                    sl = slice(jt * P, (jt + 1) * P)
                    for cc in range(2):
                        nc.sync.dma_start(cosT[:, cc, sl], d_cosetT[cc][:, sl])
                    ps = psump.tile([P, 2 * C], f32, tag="ps")
                    for cc in range(2):
                        nc.tensor.matmul(ps[:],
                                         lhsT=cosT[:, cc, sl],
                                         rhs=wkvT[:, cc, :],
                                         start=(cc == 0), stop=(cc == 1))
                    kv_sb = evacp.tile([P, 2 * C], bf16, tag="kvevac")
                    nc.scalar.copy(kv_sb[:], ps[:])
                    nc.scalar.dma_start(kvdr[jt * P:(jt + 1) * P, :], kv_sb[:])

                # index list + pg rows (needed by the first gather / t2 pass)
                idxw = constp.tile([P, QB, M * P // 16], i16)
                for g in range(8):
                    nc.sync.dma_start(idxw[g * 16:(g + 1) * 16, :, :],
                                      d_idxw16.ap())
                pgg = constp.tile([P, QB, M, POS], bf16)
                nc.sync.dma_start(pgg[:], d_pgrows.ap())
                wqT = constp.tile([P, 2, C], bf16)
                woT = constp.tile([P, 2, C], bf16)
                wlBD = constp.tile([P, 2, H * POS], bf16)
                cosQT = prepp.tile([P, 2, NQ], bf16)
                for cc in range(2):
                    nc.sync.dma_start(wqT[:, cc, :], d_wqT[cc])
                    nc.sync.dma_start(wlBD[:, cc, :], d_wlBD[cc])
                    nc.sync.dma_start(cosQT[:, cc, :], d_cosetQT[cc])
                    nc.sync.dma_start(woT[:, cc, :], d_woT[cc])
                bqv = constp.tile([P, 2, 1], f32)
                nc.sync.dma_start(bqv[:],
                                  d_bqv.ap().rearrange("c p one -> p c one"))
                brows = constp.tile([1, 3, C], bf16)
                nc.sync.dma_start(brows[:], d_brows.ap())
                identb = constp.tile([P, P], bf16)
                from concourse.masks import make_identity
                make_identity(nc, identb[:])
                ones1 = constp.tile([1, P], bf16)
                nc.vector.memset(ones1[:], 1.0)

                # G rows first (the hoisted t2 pass consumes them):
                # g = (Q + v) @ W_l (block-diagonal per head)
                qvT = prepp.tile([P, 2, NQ], bf16)
                for cc2 in range(2):
                    ps = psump.tile([P, NQ], f32, tag="ps3")
                    for cc in range(2):
                        nc.tensor.matmul(ps[:],
                                         lhsT=wqT[:, cc, cc2 * P:(cc2 + 1) * P],
                                         rhs=cosQT[:, cc, :],
                                         start=(cc == 0), stop=(cc == 1))
                    nc.vector.tensor_tensor(
                        out=qvT[:, cc2, :], in0=ps[:],
                        in1=bqv[:, cc2, :].broadcast_to([P, NQ]), op=add)
                for nt in range(QB):
                    ps = psump.tile([P, H * POS], f32, tag="ps4")
                    for cc in range(2):
                        nc.tensor.matmul(ps[:],
                                         lhsT=qvT[:, cc, nt * P:(nt + 1) * P],
                                         rhs=wlBD[:, cc, :],
                                         start=(cc == 0), stop=(cc == 1))
                    nc.scalar.copy(g_rows[:, nt, :], ps[:])

                # Q rows (bf16, b_q added via PE ones outer product)
                for nt in range(QB):
                    ps = psump.tile([P, C], f32, tag="ps2")
                    for cc in range(2):
                        nc.tensor.matmul(ps[:],
                                         lhsT=cosQT[:, cc, nt * P:(nt + 1) * P],
                                         rhs=wqT[:, cc, :],
                                         start=(cc == 0), stop=False)
                    nc.tensor.matmul(ps[:], lhsT=ones1[:],
                                     rhs=brows[:, 0, :],
                                     start=False, stop=True)
                    nc.scalar.copy(q_rows[:, nt, :], ps[:])

                # bias broadcast mats via PE ones outer product (epilogue-only)
                for bi, bmat in ((1, binmat), (2, boutmat)):
                    ps = psump.tile([P, C], f32, tag="ps2")
                    nc.tensor.matmul(ps[:], lhsT=ones1[:],
                                     rhs=brows[:, bi, :],
                                     start=True, stop=True)
                    nc.scalar.copy(bmat[:], ps[:])

            # ---- main loop: per query block, per m-half ----
            with (
                tc.tile_pool(name="gath", bufs=3) as gathp,
                tc.tile_pool(name="t2p", bufs=2) as t2p,
                tc.tile_pool(name="e3p", bufs=2) as e3p,
                tc.tile_pool(name="Ap", bufs=QB) as Apool,
                tc.tile_pool(name="small", bufs=2) as smallp,
                tc.tile_pool(name="epi", bufs=1) as epip,
                tc.tile_pool(name="psumM", bufs=2, space="PSUM") as psump,
            ):
              # --- hoisted t2 pass: DVE fills the gather ramp-up with the
              # pairwise-g term for ALL blocks (depends only on pgg+g_rows) ---
              As = []
              for qb in range(QB):
                A = Apool.tile([P, M, H], f32, tag="A")
                As.append(A)
                for mh in range(2):
                    ms = slice(mh * MH, (mh + 1) * MH)
                    t2t = t2p.tile([P, MH, H, POS], bf16, tag="t2t")
                    g_bc = (g_rows[:, qb, :]
                            .rearrange("p (h pp) -> p h pp", pp=POS)
                            [:, None, :, :].broadcast_to([P, MH, H, POS]))
                    pg_bc = (pgg[:, qb, ms, None, :]
                             .broadcast_to([P, MH, H, POS]))
                    nc.vector.tensor_tensor(out=t2t[:], in0=pg_bc,
                                            in1=g_bc, op=mult)
                    nc.vector.tensor_reduce(
                        out=A[:, ms, :].rearrange("p m h -> p (m h)"),
                        in_=t2t[:].rearrange("p m h pp -> p (m h) pp"),
                        axis=mybir.AxisListType.X, op=add)

              for qb in range(QB):
                A = As[qb]

                # --- KV gathers per m-half (1024-idx calls, 4 queues) ---
                kvgs = []
                for mh in range(2):
                    kvg = gathp.tile([P, MH, 2 * C], bf16, tag="kvg")
                    kvgs.append(kvg)
                    for k in range(MH * P // NIC):
                        ci = qb * 4 + mh * 2 + k  # global call index
                        nc.gpsimd.dma_gather(
                            out_ap=kvg[:, k * (NIC // P):(k + 1) * (NIC // P),
                                       :],
                            in_ap=kvdr[:],
                            idxs_ap=idxw_slice(idxw, qb,
                                               mh * (MH * P // NIC) + k, NIC),
                            num_idxs=NIC, num_idxs_reg=NIC, elem_size=2 * C,
                            single_packet=False, queue_num=ci % 4)

                # --- t1 scores + exp per m-half (Act runs ahead) ---
                E = smallp.tile([P, M, H], bf16, tag="E")
                e3s = []
                for mh in range(2):
                    kvg = kvgs[mh]
                    ms = slice(mh * MH, (mh + 1) * MH)
                    # t1 product in-place on the K half (bf16 2x mode)
                    kh = kvg[:, :, 0:C]
                    nc.vector.tensor_tensor(
                        out=kh, in0=kh,
                        in1=q_rows[:, qb, :][:, None, :].broadcast_to(
                            [P, MH, C]),
                        op=mult)
                    # d-tree: sum groups of 32 (head dim), in-place to 1 col
                    kh4 = kvg[:, :, 0:C].rearrange("p m (h d) -> p m h d", d=D)
                    half = D // 2
                    while half >= 1:
                        nc.vector.tensor_tensor(
                            out=kh4[:, :, :, 0:half],
                            in0=kh4[:, :, :, 0:half],
                            in1=kh4[:, :, :, half:2 * half], op=add)
                        half //= 2
                    # A[:, mh-slice, :] += t1 (strided bf16 view into f32 A)
                    nc.vector.tensor_tensor(
                        out=A[:, ms, :], in0=A[:, ms, :],
                        in1=kh4[:, :, :, 0], op=add)
                    # exp of this half (Act); E stays unnormalized
                    nc.scalar.activation(
                        out=E[:, ms, :], in_=A[:, ms, :],
                        func=mybir.ActivationFunctionType.Exp,
                        scale=INV_SQRT_D)
                    # expand E -> [P, MH, C] on Act (overlaps next DVE work)
                    e3 = e3p.tile([P, MH, C], bf16, tag="e3")
                    nc.scalar.copy(
                        e3[:].rearrange("p m (h d) -> p m h d", d=D),
                        E[:, ms, :, None].broadcast_to([P, MH, H, D]))
                    e3s.append(e3)

                # --- aggregation per m-half ---
                oa = epip.tile([P, 2, C], f32, tag="oa")
                for mh in range(2):
                    vh = kvgs[mh][:, :, C:2 * C]
                    nc.vector.tensor_tensor(out=vh, in0=vh, in1=e3s[mh][:],
                                            op=mult)
                    half = MH // 2
                    while half >= 2:
                        nc.vector.tensor_tensor(
                            out=vh[:, 0:half, :], in0=vh[:, 0:half, :],
                            in1=vh[:, half:2 * half, :], op=add)
                        half //= 2
                    nc.vector.tensor_tensor(out=oa[:, mh, :],
                                            in0=vh[:, 0, :], in1=vh[:, 1, :],
                                            op=add)

                # --- softmax denominator: one strided reduce over m ---
                z = epip.tile([P, H], f32, tag="z")
                nc.vector.tensor_reduce(
                    out=z[:], in_=E[:].transpose([0, 2, 1]),
                    axis=mybir.AxisListType.X, op=add)
                rz = epip.tile([P, H], f32, tag="rz")
                nc.vector.reciprocal(rz[:], z[:])

                # --- epilogue: combine halves, scale by 1/Z, + b_in ---
                agg = epip.tile([P, C], f32, tag="agg")
                nc.vector.tensor_tensor(out=agg[:], in0=oa[:, 0, :],
                                        in1=oa[:, 1, :], op=add)
                agv = agg[:].rearrange("p (h d) -> p h d", d=D)
                nc.vector.tensor_tensor(
                    out=agv, in0=agv,
                    in1=rz[:, :, None].broadcast_to([P, H, D]), op=mult)
                aggb = epip.tile([P, C], bf16, tag="aggb")
                nc.vector.tensor_tensor(out=aggb[:], in0=agg[:],
                                        in1=binmat[:], op=add)

                # --- output projection: out = (agg+b_in) @ W_out^T + b_out ---
                aggT = epip.tile([P, 2, P], bf16, tag="aggT")
                for cc in range(2):
                    pst = psump.tile([P, P], bf16, tag="pst")
                    nc.tensor.transpose(pst[:], aggb[:, cc * P:(cc + 1) * P],
                                        identb[:])
                    nc.scalar.copy(aggT[:, cc, :], pst[:])
                psO = psump.tile([P, C], f32, tag="psO")
                for cc in range(2):
                    nc.tensor.matmul(psO[:], lhsT=aggT[:, cc, :],
                                     rhs=woT[:, cc, :],
                                     start=(cc == 0), stop=(cc == 1))
                out_sb = epip.tile([P, C], f32, tag="outsb")
                nc.vector.tensor_tensor(out=out_sb[:], in0=psO[:],
                                        in1=boutmat[:], op=add)
                nc.sync.dma_start(d_out[qb], out_sb[:])

    nc.compile()
    return nc


def idxw_slice(idxw_tile, qb, k, nic):
    """Column slice of the wrapped idx tile for gather chunk k (nic idxs)."""
    ncols = nic // 16
    return idxw_tile[:, qb, k * ncols:(k + 1) * ncols]


def _wrap_idx16(lst):
    """int16 list -> [16, len/16] wrapped (pos i -> [i%16, i//16])."""
    n = lst.shape[0]
    return np.ascontiguousarray(lst.reshape(n // 16, 16).T)


def _bf16(x):
    import ml_dtypes
    return np.ascontiguousarray(np.asarray(x, np.float32)
                                .astype(ml_dtypes.bfloat16))


def make_core_inputs(pairwise_g, coset_functions, nbhd_idx,
                     W_q, b_q, W_k, W_l, u, v, W_in, b_in, W_out, b_out):
    pairwise_g = np.asarray(pairwise_g)
    coset_functions = np.asarray(coset_functions)
    nbhd_idx = np.asarray(nbhd_idx)
    W_q = np.asarray(W_q, np.float32)
    W_k = np.asarray(W_k, np.float32)
    W_l = np.asarray(W_l, np.float32)
    W_in = np.asarray(W_in, np.float32)
    W_out = np.asarray(W_out, np.float32)
    b_q = np.asarray(b_q, np.float32)
    b_in = np.asarray(b_in, np.float32)
    b_out = np.asarray(b_out, np.float32)
    v = np.asarray(v, np.float32)

    wqT = _bf16(W_q.T.reshape(2, P, C))
    # combined K || V rows (V = W_in projection; W_out applied on device)
    wkvT = _bf16(np.concatenate([W_k.T, W_in.T], axis=1).reshape(2, P, 2 * C))
    woT = _bf16(W_out.T.reshape(2, P, C))
    wlBD_full = np.zeros((C, H * POS), np.float32)
    for h in range(H):
        wlBD_full[h * D:(h + 1) * D, h * POS:(h + 1) * POS] = \
            W_l[h * D:(h + 1) * D, :]
    wlBD = _bf16(wlBD_full.reshape(2, P, H * POS))
    bqv = np.ascontiguousarray(
        (b_q + v.reshape(C)).reshape(2, P, 1).astype(np.float32))
    brows = _bf16(np.stack([b_q, b_in, b_out])[None, :, :])

    in_maps = []
    for core in range(NCORES):
        b = core // 4
        qs = (core % 4) * NQ
        cosetT = _bf16(coset_functions[b].T.reshape(2, P, N))
        cosetQT = _bf16(coset_functions[b, qs:qs + NQ].T.reshape(2, P, NQ))
        idx = nbhd_idx[b, qs:qs + NQ].astype(np.int64)  # [NQ, M]

        idxw16 = np.empty((16, QB, M * P // 16), np.int16)
        for qb in range(QB):
            blk = idx[qb * P:(qb + 1) * P]  # [P(n), M]
            # m-major list: pos i = m*128 + n
            lst = blk.T.reshape(M * P)  # [m, n] flattened
            idxw16[:, qb, :] = _wrap_idx16(lst.astype(np.int16))

        # neighborhood-gathered pairwise_g rows (host-side input sharding):
        # pgrows[n_sub, qb, m, :] = pairwise_g[b, qs+qb*128+n_sub, idx, :]
        pgr = pairwise_g[b, qs + np.arange(NQ)[:, None], idx]  # [NQ, M, POS]
        pgrows = _bf16(pgr.reshape(QB, P, M, POS).transpose(1, 0, 2, 3))

        in_maps.append({
            "cosetT": cosetT, "cosetQT": cosetQT,
            "wqT": wqT, "wkvT": wkvT, "woT": woT,
            "wlBD": wlBD, "bqv": bqv, "brows": brows,
            "idxw16": idxw16, "pgrows": pgrows,
        })
    return in_maps


def assemble_output(results):
    out = np.empty((B, N, C), np.float32)
    for core in range(NCORES):
        b = core // 4
        qs = (core % 4) * NQ
        o = results[core]["out"]  # [QB, P, C]
        out[b, qs:qs + NQ] = o.reshape(NQ, C).astype(np.float32)
    return out


def kernel(pairwise_g, coset_functions, mask, nbhd_idx,
           W_q, b_q, W_k, b_k, W_l, b_l, u, v,
           W_in, b_in, W_out, b_out, **_unused):
    from concourse.bass_utils import run_bass_kernel_spmd

    if "nc" not in _compiled:
        _compiled["nc"] = build_bass()
    nc = _compiled["nc"]

    in_maps = make_core_inputs(pairwise_g, coset_functions, nbhd_idx,
                               W_q, b_q, W_k, W_l, u, v, W_in, b_in,
                               W_out, b_out)
    res = run_bass_kernel_spmd(nc, in_maps, core_ids=list(range(NCORES)))
    return assemble_output(res.results)


# revision 22
# speedup vs baseline: 1.1418x; 1.1418x over previous
"""Trainium2 Bass kernel for nn_EquivairantMultiheadAttention (sparse attention).

Contract: kernel(**inputs) takes the FULL unsharded numpy inputs (as produced by
setup_inputs()) and returns the FULL (B, N, COUT) float32 output.

Sharding: 8 cores = data-parallel over batch (2) x sequence-parallel over the
query dim n (4 slices of 512). Each core receives its batch's coset_functions
(transposed, bf16), its query-slice of pairwise_g rows (pre-gathered by
neighborhood on the host as part of input sharding) and nbhd_idx, plus all
weights.

Math notes (equivalences used, all exact):
 - t3 = einsum(K, u) and every other per-(n,h) constant (b_k.Q, b_l.(Q+v))
   is constant across the softmax axis m, hence drops out of softmax.
 - e . (Q+v) = pg . G with G[n,h,:] = W_l[h-block,:]^T (Q+v)[n,h-block].
 - b_in is added after the weighted sum (weights sum to 1); the W_out
   projection runs on the PE per query block (it cannot be folded into V:
   attention weights differ per head while W_out mixes channels across heads).
 - mask is all ones (spec: fill=ones) -> masking is a no-op.

Performance structure:
 - Host ships the MINIMUM bytes per core (~2.5MB): coset (bf16, transposed),
   weights (bf16), the 16-partition wrapped neighbor index list (replicated
   to 128 partitions on device), and the neighborhood-gathered pairwise_g
   rows (bf16, 12B each). No dense pairwise_g and no [P,C] bias broadcasts
   cross the host link; bias rows are outer-product-broadcast on the PE.
 - K and V2 rows are built once (PE matmuls) as ONE combined bf16 row
   [K(256)||V2(256)] = 1KB and gathered with a single dma_gather per chunk.
   The dma_gather descriptor generation (SWDGE ucode on the Pool engine,
   ~8.5ns/row serialized) is the device-side critical path; everything else
   overlaps under it.
 - All pair-stream elementwise work is bf16 with unit innermost stride so the
   DVE 2x_1p mode applies; grouped reductions are add-trees of TensorTensor
   ops (TensorReduce has no fast mode). Products and trees run IN-PLACE in
   the gather buffer (SBUF pressure).
 - exp and the E->[P,MQ,C] head-broadcast run on the idle Activation engine,
   split per m-quarter so their latency hides behind later quarters' DVE work.
 - E stays UNNORMALIZED; out is scaled by 1/Z per head in the f32 epilogue.
 - KV gathers are tiled per m-QUARTER (16 neighbors, 16KB tiles, 5 buffers):
   finer tiles start t1 sooner after the K||V table is built and free slots
   earlier, so the in-order Pool engine can issue the next query block's
   gathers while this block is still reducing.
"""

import math
import sys

import numpy as np

sys.path.insert(0, "/opt/trn_rl_repo")

B, N, M = 2, 2048, 64
C = 256  # CIN == COUT
H, D, POS = 8, 32, 6
NQ = 512  # queries per core
QB = 4  # query blocks of 128 per core
P = 128
NCORES = 8
INV_SQRT_D = 1.0 / math.sqrt(D)
NIC = 1024  # max idxs per dma_gather call (HW: larger fails the exec unit)
MH = M // 2  # m-half

_compiled = {}


def build_bass():
    import concourse.bacc as bacc
    import concourse.mybir as mybir
    import concourse.tile as tile

    dt = mybir.dt
    nc = bacc.Bacc("TRN2", target_bir_lowering=False, debug=False,
                   enable_asserts=False, num_devices=NCORES,
                   num_swdge_queues=4, dynamic_dma_scratch_size=1 << 14)

    f32 = dt.float32
    bf16 = dt.bfloat16
    i16 = dt.int16

    # ---- DRAM inputs (per core) ----
    d_cosetT = nc.dram_tensor("cosetT", [2, P, N], bf16, kind="ExternalInput")
    d_cosetQT = nc.dram_tensor("cosetQT", [2, P, NQ], bf16,
                               kind="ExternalInput")
    d_wqT = nc.dram_tensor("wqT", [2, P, C], bf16, kind="ExternalInput")
    d_wkvT = nc.dram_tensor("wkvT", [2, P, 2 * C], bf16, kind="ExternalInput")
    d_woT = nc.dram_tensor("woT", [2, P, C], bf16, kind="ExternalInput")
    d_wlBD = nc.dram_tensor("wlBD", [2, P, H * POS], bf16,
                            kind="ExternalInput")
    d_bqv = nc.dram_tensor("bqv", [2, P, 1], f32, kind="ExternalInput")
    # bias rows (broadcast to [P, C] on device via ones outer product)
    d_brows = nc.dram_tensor("brows", [1, 3, C], bf16, kind="ExternalInput")
    # wrapped int16 index list (m-major: list pos i = m*128 + n_sub),
    # pre-replicated to the 8 groups of 16 partitions (one fast DMA)
    d_idxw = nc.dram_tensor("idxw", [P, QB, M * P // 16], i16,
                            kind="ExternalInput")
    # neighborhood-gathered pairwise_g rows, bf16: [n_sub, qb, m, POS]
    d_pgrows = nc.dram_tensor("pgrows", [P, QB, M, POS], bf16,
                              kind="ExternalInput")
    d_out = nc.dram_tensor("out", [QB, P, C], f32, kind="ExternalOutput")

    add = mybir.AluOpType.add
    mult = mybir.AluOpType.mult

    with tile.TileContext(nc) as tc:
        with (
            tc.tile_pool(name="const", bufs=1) as constp,
            tc.tile_pool(name="dram", bufs=1, space="DRAM") as dramp,
            tc.tile_pool(name="evac", bufs=2) as evacp,
        ):
            # critical-path loads first: the qb0 KV gathers gate only on
            # idxw + the kvdr table (cosT + wkvT -> PE build)
            wkvT = constp.tile([P, 2, 2 * C], bf16)
            for cc in range(2):
                nc.sync.dma_start(wkvT[:, cc, :], d_wkvT[cc])

            # combined K||V2 rows in DRAM scratch (bf16, 1KB rows)
            kvdr = dramp.tile([N, 2 * C], bf16)

            q_rows = constp.tile([P, QB, C], bf16)
            g_rows = constp.tile([P, QB, H * POS], bf16)
            binmat = constp.tile([P, C], f32)
            boutmat = constp.tile([P, C], f32)

            with (
                tc.tile_pool(name="prep", bufs=1) as prepp,
                tc.tile_pool(name="psumP", bufs=2, space="PSUM") as psump,
            ):
                # K||V build first: the gathers gate on the full kvdr table.
                # cosT arrives in per-slab column chunks so slab jt's matmul
                # starts as soon as its own 64KB landed, not after the full
                # 1MB load.
                # K||V build first: cosT arrives in 4 column chunks (per-DMA
                # issue overhead is ~0.6us, so fewer+bigger beats per-slab).
                # kvdr writebacks are grouped 4 slabs per DMA and go out on
                # the SECOND HWDGE queue (scalar) so they don't head-of-line
                # block later loads on sync.
                cosT = prepp.tile([P, 2, N], bf16)
                SG = 4  # slabs per cosT chunk / per kvdr writeback
                for g in range(N // P // SG):
                    sl = slice(g * SG * P, (g + 1) * SG * P)
                    for cc in range(2):
                        nc.sync.dma_start(cosT[:, cc, sl], d_cosetT[cc][:, sl])
                    kvw = evacp.tile([P, SG, 2 * C], bf16, tag="kvevac")
                    for j2 in range(SG):
                        jt = g * SG + j2
                        ps = psump.tile([P, 2 * C], f32, tag="ps")
                        for cc in range(2):
                            nc.tensor.matmul(
                                ps[:],
                                lhsT=cosT[:, cc, jt * P:(jt + 1) * P],
                                rhs=wkvT[:, cc, :],
                                start=(cc == 0), stop=(cc == 1))
                        nc.scalar.copy(kvw[:, j2, :], ps[:])
                    nc.scalar.dma_start(
                        kvdr[g * SG * P:(g + 1) * SG * P, :]
                        .rearrange("(s p) c -> p s c", s=SG),
                        kvw[:])

                # index list + pg rows (needed by the first gather / t2)
                idxw = constp.tile([P, QB, M * P // 16], i16)
                nc.sync.dma_start(idxw[:], d_idxw.ap())
                pgg = constp.tile([P, QB, M, POS], bf16)
                nc.sync.dma_start(pgg[:], d_pgrows.ap())
                wqT = constp.tile([P, 2, C], bf16)
                woT = constp.tile([P, 2, C], bf16)
                wlBD = constp.tile([P, 2, H * POS], bf16)
                cosQT = prepp.tile([P, 2, NQ], bf16)
                for cc in range(2):
                    nc.sync.dma_start(wqT[:, cc, :], d_wqT[cc])
                    nc.sync.dma_start(wlBD[:, cc, :], d_wlBD[cc])
                    nc.sync.dma_start(cosQT[:, cc, :], d_cosetQT[cc])
                    nc.sync.dma_start(woT[:, cc, :], d_woT[cc])
                bqv = constp.tile([P, 2, 1], f32)
                nc.sync.dma_start(bqv[:],
                                  d_bqv.ap().rearrange("c p one -> p c one"))
                brows = constp.tile([1, 3, C], bf16)
                nc.sync.dma_start(brows[:], d_brows.ap())
                identb = constp.tile([P, P], bf16)
                from concourse.masks import make_identity
                make_identity(nc, identb[:])
                ones1 = constp.tile([1, P], bf16)
                nc.vector.memset(ones1[:], 1.0)

                # G rows first (the hoisted t2 pass consumes them):
                # g = (Q + v) @ W_l (block-diagonal per head)
                qvT = prepp.tile([P, 2, NQ], bf16)
                for cc2 in range(2):
                    ps = psump.tile([P, NQ], f32, tag="ps3")
                    for cc in range(2):
                        nc.tensor.matmul(ps[:],
                                         lhsT=wqT[:, cc, cc2 * P:(cc2 + 1) * P],
                                         rhs=cosQT[:, cc, :],
                                         start=(cc == 0), stop=(cc == 1))
                    nc.vector.tensor_tensor(
                        out=qvT[:, cc2, :], in0=ps[:],
                        in1=bqv[:, cc2, :].broadcast_to([P, NQ]), op=add)
                for nt in range(QB):
                    ps = psump.tile([P, H * POS], f32, tag="ps4")
                    for cc in range(2):
                        nc.tensor.matmul(ps[:],
                                         lhsT=qvT[:, cc, nt * P:(nt + 1) * P],
                                         rhs=wlBD[:, cc, :],
                                         start=(cc == 0), stop=(cc == 1))
                    nc.scalar.copy(g_rows[:, nt, :], ps[:])

                # Q rows (bf16, b_q added via PE ones outer product)
                for nt in range(QB):
                    ps = psump.tile([P, C], f32, tag="ps2")
                    for cc in range(2):
                        nc.tensor.matmul(ps[:],
                                         lhsT=cosQT[:, cc, nt * P:(nt + 1) * P],
                                         rhs=wqT[:, cc, :],
                                         start=(cc == 0), stop=False)
                    nc.tensor.matmul(ps[:], lhsT=ones1[:],
                                     rhs=brows[:, 0, :],
                                     start=False, stop=True)
                    nc.scalar.copy(q_rows[:, nt, :], ps[:])

                # bias broadcast mats via PE ones outer product (epilogue-only)
                for bi, bmat in ((1, binmat), (2, boutmat)):
                    ps = psump.tile([P, C], f32, tag="ps2")
                    nc.tensor.matmul(ps[:], lhsT=ones1[:],
                                     rhs=brows[:, bi, :],
                                     start=True, stop=True)
                    nc.scalar.copy(bmat[:], ps[:])

            # ---- main loop: per query block, per m-half ----
            with (
                tc.tile_pool(name="gath", bufs=3) as gathp,
                tc.tile_pool(name="t2p", bufs=2) as t2p,
                tc.tile_pool(name="e3p", bufs=2) as e3p,
                tc.tile_pool(name="q3p", bufs=1) as q3p,
                tc.tile_pool(name="small", bufs=2) as smallp,
                tc.tile_pool(name="epi", bufs=1) as epip,
                tc.tile_pool(name="psumM", bufs=2, space="PSUM") as psump,
            ):
              for qb in range(QB):
                A = smallp.tile([P, M, H], f32, tag="A")

                # --- KV gathers per m-half (1024-idx calls, 4 queues) ---
                kvgs = []
                for mh in range(2):
                    kvg = gathp.tile([P, MH, 2 * C], bf16, tag="kvg")
                    kvgs.append(kvg)
                    for k in range(MH * P // NIC):
                        ci = qb * 4 + mh * 2 + k  # global call index
                        nc.gpsimd.dma_gather(
                            out_ap=kvg[:, k * (NIC // P):(k + 1) * (NIC // P),
                                       :],
                            in_ap=kvdr[:],
                            idxs_ap=idxw_slice(idxw, qb,
                                               mh * (MH * P // NIC) + k, NIC),
                            num_idxs=NIC, num_idxs_reg=NIC, elem_size=2 * C,
                            single_packet=False, queue_num=ci % 4)

                # Q broadcast materialized on Act: a unit-stride in1 keeps the
                # DVE t1 product in fast mode (broadcast-AP in1 halves it)
                q3 = q3p.tile([P, MH, C], bf16, tag="q3")
                nc.scalar.copy(
                    q3[:], q_rows[:, qb, :][:, None, :].broadcast_to(
                        [P, MH, C]))

                # --- t2 product (bf16 2x on DVE; overlaps the gathers) ---
                for mh in range(2):
                    ms = slice(mh * MH, (mh + 1) * MH)
                    t2t = t2p.tile([P, MH, H, POS], bf16, tag="t2t")
                    g_bc = (g_rows[:, qb, :]
                            .rearrange("p (h pp) -> p h pp", pp=POS)
                            [:, None, :, :].broadcast_to([P, MH, H, POS]))
                    pg_bc = (pgg[:, qb, ms, None, :]
                             .broadcast_to([P, MH, H, POS]))
                    nc.vector.tensor_tensor(out=t2t[:], in0=pg_bc,
                                            in1=g_bc, op=mult)
                    nc.vector.tensor_reduce(
                        out=A[:, ms, :].rearrange("p m h -> p (m h)"),
                        in_=t2t[:].rearrange("p m h pp -> p (m h) pp"),
                        axis=mybir.AxisListType.X, op=add)

                # --- t1 scores + exp per m-half (Act runs ahead) ---
                E = smallp.tile([P, M, H], bf16, tag="E")
                e3s = []
                for mh in range(2):
                    kvg = kvgs[mh]
                    ms = slice(mh * MH, (mh + 1) * MH)
                    # t1 product in-place on the K half (bf16 2x mode)
                    kh = kvg[:, :, 0:C]
                    nc.vector.tensor_tensor(out=kh, in0=kh, in1=q3[:],
                                            op=mult)
                    # d-tree: sum groups of 32 (head dim), in-place to 1 col
                    kh4 = kvg[:, :, 0:C].rearrange("p m (h d) -> p m h d", d=D)
                    half = D // 2
                    while half >= 1:
                        nc.vector.tensor_tensor(
                            out=kh4[:, :, :, 0:half],
                            in0=kh4[:, :, :, 0:half],
                            in1=kh4[:, :, :, half:2 * half], op=add)
                        half //= 2
                    # A[:, mh-slice, :] += t1 (strided bf16 view into f32 A)
                    nc.vector.tensor_tensor(
                        out=A[:, ms, :], in0=A[:, ms, :],
                        in1=kh4[:, :, :, 0], op=add)
                    # exp of this half (Act); E stays unnormalized
                    nc.scalar.activation(
                        out=E[:, ms, :], in_=A[:, ms, :],
                        func=mybir.ActivationFunctionType.Exp,
                        scale=INV_SQRT_D)
                    # expand E -> [P, MH, C] on Act (overlaps next DVE work)
                    e3 = e3p.tile([P, MH, C], bf16, tag="e3")
                    nc.scalar.copy(
                        e3[:].rearrange("p m (h d) -> p m h d", d=D),
                        E[:, ms, :, None].broadcast_to([P, MH, H, D]))
                    e3s.append(e3)

                # --- aggregation per m-half ---
                oa = epip.tile([P, 2, C], f32, tag="oa")
                for mh in range(2):
                    vh = kvgs[mh][:, :, C:2 * C]
                    nc.vector.tensor_tensor(out=vh, in0=vh, in1=e3s[mh][:],
                                            op=mult)
                    half = MH // 2
                    while half >= 2:
                        nc.vector.tensor_tensor(
                            out=vh[:, 0:half, :], in0=vh[:, 0:half, :],
                            in1=vh[:, half:2 * half, :], op=add)
                        half //= 2
                    nc.vector.tensor_tensor(out=oa[:, mh, :],
                                            in0=vh[:, 0, :], in1=vh[:, 1, :],
                                            op=add)

                # --- softmax denominator: one strided reduce over m ---
                z = epip.tile([P, H], f32, tag="z")
                nc.vector.tensor_reduce(
                    out=z[:], in_=E[:].transpose([0, 2, 1]),
                    axis=mybir.AxisListType.X, op=add)
                rz = epip.tile([P, H], f32, tag="rz")
                nc.vector.reciprocal(rz[:], z[:])

                # --- epilogue: combine halves, scale by 1/Z, + b_in ---
                agg = epip.tile([P, C], f32, tag="agg")
                nc.vector.tensor_tensor(out=agg[:], in0=oa[:, 0, :],
                                        in1=oa[:, 1, :], op=add)
                agv = agg[:].rearrange("p (h d) -> p h d", d=D)
                nc.vector.tensor_tensor(
                    out=agv, in0=agv,
                    in1=rz[:, :, None].broadcast_to([P, H, D]), op=mult)
                aggb = epip.tile([P, C], bf16, tag="aggb")
                nc.vector.tensor_tensor(out=aggb[:], in0=agg[:],
                                        in1=binmat[:], op=add)

                # --- output projection: out = (agg+b_in) @ W_out^T + b_out ---
                aggT = epip.tile([P, 2, P], bf16, tag="aggT")
                for cc in range(2):
                    pst = psump.tile([P, P], bf16, tag="pst")
                    nc.tensor.transpose(pst[:], aggb[:, cc * P:(cc + 1) * P],
                                        identb[:])
                    nc.scalar.copy(aggT[:, cc, :], pst[:])
                psO = psump.tile([P, C], f32, tag="psO")
                for cc in range(2):
                    nc.tensor.matmul(psO[:], lhsT=aggT[:, cc, :],
                                     rhs=woT[:, cc, :],
                                     start=(cc == 0), stop=(cc == 1))
                out_sb = epip.tile([P, C], f32, tag="outsb")
                nc.vector.tensor_tensor(out=out_sb[:], in0=psO[:],
                                        in1=boutmat[:], op=add)
                nc.sync.dma_start(d_out[qb], out_sb[:])

    nc.compile()
    return nc


def idxw_slice(idxw_tile, qb, k, nic):
    """Column slice of the wrapped idx tile for gather chunk k (nic idxs)."""
    ncols = nic // 16
    return idxw_tile[:, qb, k * ncols:(k + 1) * ncols]


def _wrap_idx(lst):
    """int16 list -> [128, len/16] wrapped (pos i -> [i%16, i//16]) and
    replicated across the 8 groups of 16 partitions."""
    n = lst.shape[0]
    w = np.empty((P, n // 16), np.int16)
    blk = lst.reshape(n // 16, 16).T  # [16, n/16]
    for g in range(8):
        w[g * 16:(g + 1) * 16, :] = blk
    return w


def _bf16(x):
    import ml_dtypes
    return np.ascontiguousarray(np.asarray(x, np.float32)
                                .astype(ml_dtypes.bfloat16))


def make_core_inputs(pairwise_g, coset_functions, nbhd_idx,
                     W_q, b_q, W_k, W_l, u, v, W_in, b_in, W_out, b_out):
    pairwise_g = np.asarray(pairwise_g)
    coset_functions = np.asarray(coset_functions)
    nbhd_idx = np.asarray(nbhd_idx)
    W_q = np.asarray(W_q, np.float32)
    W_k = np.asarray(W_k, np.float32)
    W_l = np.asarray(W_l, np.float32)
    W_in = np.asarray(W_in, np.float32)
    W_out = np.asarray(W_out, np.float32)
    b_q = np.asarray(b_q, np.float32)
    b_in = np.asarray(b_in, np.float32)
    b_out = np.asarray(b_out, np.float32)
    v = np.asarray(v, np.float32)

    wqT = _bf16(W_q.T.reshape(2, P, C))
    # combined K || V rows (V = W_in projection; W_out applied on device)
    wkvT = _bf16(np.concatenate([W_k.T, W_in.T], axis=1).reshape(2, P, 2 * C))
    woT = _bf16(W_out.T.reshape(2, P, C))
    wlBD_full = np.zeros((C, H * POS), np.float32)
    for h in range(H):
        wlBD_full[h * D:(h + 1) * D, h * POS:(h + 1) * POS] = \
            W_l[h * D:(h + 1) * D, :]
    wlBD = _bf16(wlBD_full.reshape(2, P, H * POS))
    bqv = np.ascontiguousarray(
        (b_q + v.reshape(C)).reshape(2, P, 1).astype(np.float32))
    brows = _bf16(np.stack([b_q, b_in, b_out])[None, :, :])

    in_maps = []
    for core in range(NCORES):
        b = core // 4
        qs = (core % 4) * NQ
        cosetT = _bf16(coset_functions[b].T.reshape(2, P, N))
        cosetQT = _bf16(coset_functions[b, qs:qs + NQ].T.reshape(2, P, NQ))
        idx = nbhd_idx[b, qs:qs + NQ].astype(np.int64)  # [NQ, M]

        idxw = np.empty((P, QB, M * P // 16), np.int16)
        for qb in range(QB):
            blk = idx[qb * P:(qb + 1) * P]  # [P(n), M]
            # m-major list: pos i = m*128 + n
            lst = blk.T.reshape(M * P)  # [m, n] flattened
            idxw[:, qb, :] = _wrap_idx(lst.astype(np.int16))

        # neighborhood-gathered pairwise_g rows (host-side input sharding):
        # pgrows[n_sub, qb, m, :] = pairwise_g[b, qs+qb*128+n_sub, idx, :]
        pgr = pairwise_g[b, qs + np.arange(NQ)[:, None], idx]  # [NQ, M, POS]
        pgrows = _bf16(pgr.reshape(QB, P, M, POS).transpose(1, 0, 2, 3))

        in_maps.append({
            "cosetT": cosetT, "cosetQT": cosetQT,
            "wqT": wqT, "wkvT": wkvT, "woT": woT,
            "wlBD": wlBD, "bqv": bqv, "brows": brows,
            "idxw": idxw, "pgrows": pgrows,
        })
    return in_maps


def assemble_output(results):
    out = np.empty((B, N, C), np.float32)
    for core in range(NCORES):
        b = core // 4
        qs = (core % 4) * NQ
        o = results[core]["out"]  # [QB, P, C]
        out[b, qs:qs + NQ] = o.reshape(NQ, C).astype(np.float32)
    return out


def kernel(pairwise_g, coset_functions, mask, nbhd_idx,
           W_q, b_q, W_k, b_k, W_l, b_l, u, v,
           W_in, b_in, W_out, b_out, **_unused):
    from concourse.bass_utils import run_bass_kernel_spmd

    if "nc" not in _compiled:
        _compiled["nc"] = build_bass()
    nc = _compiled["nc"]

    in_maps = make_core_inputs(pairwise_g, coset_functions, nbhd_idx,
                               W_q, b_q, W_k, W_l, u, v, W_in, b_in,
                               W_out, b_out)
    res = run_bass_kernel_spmd(nc, in_maps, core_ids=list(range(NCORES)))
    return assemble_output(res.results)


# revision 24
# speedup vs baseline: 1.1726x; 1.0270x over previous
"""Trainium2 Bass kernel for nn_EquivairantMultiheadAttention (sparse attention).

Contract: kernel(**inputs) takes the FULL unsharded numpy inputs (as produced by
setup_inputs()) and returns the FULL (B, N, COUT) float32 output.

Sharding: 8 cores = data-parallel over batch (2) x sequence-parallel over the
query dim n (4 slices of 512). Each core receives its batch's coset_functions
(transposed, bf16), its query-slice of pairwise_g rows (pre-gathered by
neighborhood on the host as part of input sharding) and nbhd_idx, plus all
weights.

Math notes (equivalences used, all exact):
 - t3 = einsum(K, u) and every other per-(n,h) constant (b_k.Q, b_l.(Q+v))
   is constant across the softmax axis m, hence drops out of softmax.
 - e . (Q+v) = pg . G with G[n,h,:] = W_l[h-block,:]^T (Q+v)[n,h-block].
 - b_in is added after the weighted sum (weights sum to 1); the W_out
   projection runs on the PE per query block (it cannot be folded into V:
   attention weights differ per head while W_out mixes channels across heads).
 - mask is all ones (spec: fill=ones) -> masking is a no-op.

Performance structure:
 - Host ships the MINIMUM bytes per core (~2.5MB): coset (bf16, transposed),
   weights (bf16), the 16-partition wrapped neighbor index list (replicated
   to 128 partitions on device), and the neighborhood-gathered pairwise_g
   rows (bf16, 12B each). No dense pairwise_g and no [P,C] bias broadcasts
   cross the host link; bias rows are outer-product-broadcast on the PE.
 - K and V2 rows are built once (PE matmuls) as ONE combined bf16 row
   [K(256)||V2(256)] = 1KB and gathered with a single dma_gather per chunk.
   The dma_gather descriptor generation (SWDGE ucode on the Pool engine,
   ~8.5ns/row serialized) is the device-side critical path; everything else
   overlaps under it.
 - All pair-stream elementwise work is bf16 with unit innermost stride so the
   DVE 2x_1p mode applies; grouped reductions are add-trees of TensorTensor
   ops (TensorReduce has no fast mode). Products and trees run IN-PLACE in
   the gather buffer (SBUF pressure).
 - exp and the E->[P,MQ,C] head-broadcast run on the idle Activation engine,
   split per m-quarter so their latency hides behind later quarters' DVE work.
 - E stays UNNORMALIZED; out is scaled by 1/Z per head in the f32 epilogue.
 - KV gathers are tiled per m-QUARTER (16 neighbors, 16KB tiles, 5 buffers):
   finer tiles start t1 sooner after the K||V table is built and free slots
   earlier, so the in-order Pool engine can issue the next query block's
   gathers while this block is still reducing.
"""

import math
import sys

import numpy as np

sys.path.insert(0, "/opt/trn_rl_repo")

B, N, M = 2, 2048, 64
C = 256  # CIN == COUT
H, D, POS = 8, 32, 6
NQ = 512  # queries per core
QB = 4  # query blocks of 128 per core
P = 128
NCORES = 8
INV_SQRT_D = 1.0 / math.sqrt(D)
NIC = 1024  # max idxs per dma_gather call (HW: larger fails the exec unit)
MH = M // 2  # m-half

_compiled = {}


def build_bass():
    import concourse.bacc as bacc
    import concourse.mybir as mybir
    import concourse.tile as tile

    dt = mybir.dt
    nc = bacc.Bacc("TRN2", target_bir_lowering=False, debug=False,
                   enable_asserts=False, num_devices=NCORES,
                   num_swdge_queues=4, dynamic_dma_scratch_size=1 << 14)

    f32 = dt.float32
    bf16 = dt.bfloat16
    i16 = dt.int16

    # ---- DRAM inputs (per core) ----
    d_cosetT = nc.dram_tensor("cosetT", [2, P, N], bf16, kind="ExternalInput")
    d_cosetQT = nc.dram_tensor("cosetQT", [2, P, NQ], bf16,
                               kind="ExternalInput")
    d_wqT = nc.dram_tensor("wqT", [2, P, C], bf16, kind="ExternalInput")
    d_wkvT = nc.dram_tensor("wkvT", [2, P, 2 * C], bf16, kind="ExternalInput")
    d_woT = nc.dram_tensor("woT", [2, P, C], bf16, kind="ExternalInput")
    d_wlBD = nc.dram_tensor("wlBD", [2, P, H * POS], bf16,
                            kind="ExternalInput")
    d_bqv = nc.dram_tensor("bqv", [2, P, 1], f32, kind="ExternalInput")
    # bias rows (broadcast to [P, C] on device via ones outer product)
    d_brows = nc.dram_tensor("brows", [1, 3, C], bf16, kind="ExternalInput")
    # wrapped int16 index list (m-major: list pos i = m*128 + n_sub),
    # pre-replicated to the 8 groups of 16 partitions (one fast DMA)
    d_idxw = nc.dram_tensor("idxw", [P, QB, M * P // 16], i16,
                            kind="ExternalInput")
    # neighborhood-gathered pairwise_g rows, bf16: [n_sub, qb, m, POS]
    d_pgrows = nc.dram_tensor("pgrows", [P, QB, M, POS], bf16,
                              kind="ExternalInput")
    d_out = nc.dram_tensor("out", [QB, P, C], f32, kind="ExternalOutput")

    add = mybir.AluOpType.add
    mult = mybir.AluOpType.mult

    with tile.TileContext(nc) as tc:
        with (
            tc.tile_pool(name="const", bufs=1) as constp,
            tc.tile_pool(name="dram", bufs=1, space="DRAM") as dramp,
            tc.tile_pool(name="evac", bufs=2) as evacp,
        ):
            # critical-path loads first: the qb0 KV gathers gate only on
            # idxw + the kvdr table (cosT + wkvT -> PE build)
            wkvT = constp.tile([P, 2, 2 * C], bf16)
            for cc in range(2):
                nc.sync.dma_start(wkvT[:, cc, :], d_wkvT[cc])

            # combined K||V2 rows in DRAM scratch (bf16, 1KB rows)
            kvdr = dramp.tile([N, 2 * C], bf16)

            q_rows = constp.tile([P, QB, C], bf16)
            g_rows = constp.tile([P, QB, H * POS], bf16)
            boutmat = constp.tile([P, C], f32)

            with (
                tc.tile_pool(name="prep", bufs=1) as prepp,
                tc.tile_pool(name="psumP", bufs=2, space="PSUM") as psump,
            ):
                # K||V build first: the gathers gate on the full kvdr table.
                # cosT arrives in per-slab column chunks so slab jt's matmul
                # starts as soon as its own 64KB landed, not after the full
                # 1MB load.
                # K||V build first: cosT arrives in 4 column chunks (per-DMA
                # issue overhead is ~0.6us, so fewer+bigger beats per-slab).
                # kvdr writebacks are grouped 4 slabs per DMA and go out on
                # the SECOND HWDGE queue (scalar) so they don't head-of-line
                # block later loads on sync.
                cosT = prepp.tile([P, 2, N], bf16)
                SG = 8  # slabs per cosT chunk / per kvdr writeback
                for g in range(N // P // SG):
                    sl = slice(g * SG * P, (g + 1) * SG * P)
                    for cc in range(2):
                        nc.sync.dma_start(cosT[:, cc, sl], d_cosetT[cc][:, sl])
                    kvw = evacp.tile([P, SG, 2 * C], bf16, tag="kvevac")
                    for j2 in range(SG):
                        jt = g * SG + j2
                        ps = psump.tile([P, 2 * C], f32, tag="ps")
                        for cc in range(2):
                            nc.tensor.matmul(
                                ps[:],
                                lhsT=cosT[:, cc, jt * P:(jt + 1) * P],
                                rhs=wkvT[:, cc, :],
                                start=(cc == 0), stop=(cc == 1))
                        nc.scalar.copy(kvw[:, j2, :], ps[:])
                    nc.scalar.dma_start(
                        kvdr[g * SG * P:(g + 1) * SG * P, :]
                        .rearrange("(s p) c -> p s c", s=SG),
                        kvw[:])

                # index list + pg rows (needed by the first gather / t2)
                idxw = constp.tile([P, QB, M * P // 16], i16)
                nc.sync.dma_start(idxw[:], d_idxw.ap())
                pgg = constp.tile([P, QB, M, POS], bf16)
                nc.sync.dma_start(pgg[:], d_pgrows.ap())
                wqT = constp.tile([P, 2, C], bf16)
                woT = constp.tile([P, 2, C], bf16)
                wlBD = constp.tile([P, 2, H * POS], bf16)
                cosQT = prepp.tile([P, 2, NQ], bf16)
                for cc in range(2):
                    nc.sync.dma_start(wqT[:, cc, :], d_wqT[cc])
                    nc.sync.dma_start(wlBD[:, cc, :], d_wlBD[cc])
                    nc.sync.dma_start(cosQT[:, cc, :], d_cosetQT[cc])
                    nc.sync.dma_start(woT[:, cc, :], d_woT[cc])
                bqv = constp.tile([P, 2, 1], f32)
                nc.sync.dma_start(bqv[:],
                                  d_bqv.ap().rearrange("c p one -> p c one"))
                brows = constp.tile([1, 3, C], bf16)
                nc.sync.dma_start(brows[:], d_brows.ap())
                identb = constp.tile([P, P], bf16)
                from concourse.masks import make_identity
                make_identity(nc, identb[:])
                ones1 = constp.tile([1, P], bf16)
                nc.vector.memset(ones1[:], 1.0)

                # G rows first (the hoisted t2 pass consumes them):
                # g = (Q + v) @ W_l (block-diagonal per head)
                qvT = prepp.tile([P, 2, NQ], bf16)
                for cc2 in range(2):
                    ps = psump.tile([P, NQ], f32, tag="ps3")
                    for cc in range(2):
                        nc.tensor.matmul(ps[:],
                                         lhsT=wqT[:, cc, cc2 * P:(cc2 + 1) * P],
                                         rhs=cosQT[:, cc, :],
                                         start=(cc == 0), stop=(cc == 1))
                    nc.vector.tensor_tensor(
                        out=qvT[:, cc2, :], in0=ps[:],
                        in1=bqv[:, cc2, :].broadcast_to([P, NQ]), op=add)
                for nt in range(QB):
                    ps = psump.tile([P, H * POS], f32, tag="ps4")
                    for cc in range(2):
                        nc.tensor.matmul(ps[:],
                                         lhsT=qvT[:, cc, nt * P:(nt + 1) * P],
                                         rhs=wlBD[:, cc, :],
                                         start=(cc == 0), stop=(cc == 1))
                    nc.scalar.copy(g_rows[:, nt, :], ps[:])

                # Q rows (bf16, b_q added via PE ones outer product)
                for nt in range(QB):
                    ps = psump.tile([P, C], f32, tag="ps2")
                    for cc in range(2):
                        nc.tensor.matmul(ps[:],
                                         lhsT=cosQT[:, cc, nt * P:(nt + 1) * P],
                                         rhs=wqT[:, cc, :],
                                         start=(cc == 0), stop=False)
                    nc.tensor.matmul(ps[:], lhsT=ones1[:],
                                     rhs=brows[:, 0, :],
                                     start=False, stop=True)
                    nc.scalar.copy(q_rows[:, nt, :], ps[:])

                # bias broadcast mat via PE ones outer product (epilogue-only;
                # brows[2] = b_out + b_in @ W_out.T folded on the host)
                ps = psump.tile([P, C], f32, tag="ps2")
                nc.tensor.matmul(ps[:], lhsT=ones1[:],
                                 rhs=brows[:, 2, :],
                                 start=True, stop=True)
                nc.scalar.copy(boutmat[:], ps[:])

            # ---- main loop: per query block, per m-half ----
            with (
                tc.tile_pool(name="gath", bufs=3) as gathp,
                tc.tile_pool(name="t2p", bufs=2) as t2p,
                tc.tile_pool(name="e3p", bufs=3) as e3p,
                tc.tile_pool(name="small", bufs=2) as smallp,
                tc.tile_pool(name="epi", bufs=1) as epip,
                tc.tile_pool(name="psumM", bufs=2, space="PSUM") as psump,
            ):
              for qb in range(QB):
                A = smallp.tile([P, M, H], f32, tag="A")

                # --- KV gathers per m-half (1024-idx calls, 4 queues) ---
                kvgs = []
                for mh in range(2):
                    kvg = gathp.tile([P, MH, 2 * C], bf16, tag="kvg")
                    kvgs.append(kvg)
                    for k in range(MH * P // NIC):
                        ci = qb * 4 + mh * 2 + k  # global call index
                        nc.gpsimd.dma_gather(
                            out_ap=kvg[:, k * (NIC // P):(k + 1) * (NIC // P),
                                       :],
                            in_ap=kvdr[:],
                            idxs_ap=idxw_slice(idxw, qb,
                                               mh * (MH * P // NIC) + k, NIC),
                            num_idxs=NIC, num_idxs_reg=NIC, elem_size=2 * C,
                            single_packet=False, queue_num=ci % 4)

                # --- t2 product (bf16 2x on DVE; overlaps the gathers) ---
                for mh in range(2):
                    ms = slice(mh * MH, (mh + 1) * MH)
                    t2t = t2p.tile([P, MH, H, POS], bf16, tag="t2t")
                    g_bc = (g_rows[:, qb, :]
                            .rearrange("p (h pp) -> p h pp", pp=POS)
                            [:, None, :, :].broadcast_to([P, MH, H, POS]))
                    pg_bc = (pgg[:, qb, ms, None, :]
                             .broadcast_to([P, MH, H, POS]))
                    nc.vector.tensor_tensor(out=t2t[:], in0=pg_bc,
                                            in1=g_bc, op=mult)
                    nc.vector.tensor_reduce(
                        out=A[:, ms, :].rearrange("p m h -> p (m h)"),
                        in_=t2t[:].rearrange("p m h pp -> p (m h) pp"),
                        axis=mybir.AxisListType.X, op=add)

                # --- t1 scores + exp per m-half (Act runs ahead) ---
                # E is head-major [P, H, M] so the Z reduce streams the
                # innermost m contiguously instead of a strided transpose
                E = smallp.tile([P, H, M], bf16, tag="E")
                e3s = []
                for mh in range(2):
                    kvg = kvgs[mh]
                    ms = slice(mh * MH, (mh + 1) * MH)
                    # t1 product in-place on the K half (bf16 2x mode)
                    kh = kvg[:, :, 0:C]
                    nc.vector.tensor_tensor(
                        out=kh, in0=kh,
                        in1=q_rows[:, qb, :][:, None, :].broadcast_to(
                            [P, MH, C]),
                        op=mult)
                    # d-tree: sum groups of 32 (head dim), in-place to 1 col
                    kh4 = kvg[:, :, 0:C].rearrange("p m (h d) -> p m h d", d=D)
                    half = D // 2
                    while half >= 1:
                        nc.vector.tensor_tensor(
                            out=kh4[:, :, :, 0:half],
                            in0=kh4[:, :, :, 0:half],
                            in1=kh4[:, :, :, half:2 * half], op=add)
                        half //= 2
                    # A[:, mh-slice, :] += t1 (strided bf16 view into f32 A)
                    nc.vector.tensor_tensor(
                        out=A[:, ms, :], in0=A[:, ms, :],
                        in1=kh4[:, :, :, 0], op=add)
                    # exp of this half (Act); E stays unnormalized
                    nc.scalar.activation(
                        out=E[:, :, ms].transpose([0, 2, 1]),
                        in_=A[:, ms, :],
                        func=mybir.ActivationFunctionType.Exp,
                        scale=INV_SQRT_D)
                    # expand E -> [P, MH, C] on Act (overlaps next DVE work)
                    e3 = e3p.tile([P, MH, C], bf16, tag="e3")
                    nc.scalar.copy(
                        e3[:].rearrange("p m (h d) -> p m h d", d=D),
                        E[:, :, ms].transpose([0, 2, 1])[:, :, :, None]
                        .broadcast_to([P, MH, H, D]))
                    e3s.append(e3)

                # --- aggregation per m-half ---
                oa = epip.tile([P, 2, C], f32, tag="oa")
                for mh in range(2):
                    vh = kvgs[mh][:, :, C:2 * C]
                    nc.vector.tensor_tensor(out=vh, in0=vh, in1=e3s[mh][:],
                                            op=mult)
                    half = MH // 2
                    while half >= 2:
                        nc.vector.tensor_tensor(
                            out=vh[:, 0:half, :], in0=vh[:, 0:half, :],
                            in1=vh[:, half:2 * half, :], op=add)
                        half //= 2
                    nc.vector.tensor_tensor(out=oa[:, mh, :],
                                            in0=vh[:, 0, :], in1=vh[:, 1, :],
                                            op=add)

                # --- softmax denominator: one strided reduce over m ---
                z = epip.tile([P, H], f32, tag="z")
                nc.vector.tensor_reduce(
                    out=z[:], in_=E[:],
                    axis=mybir.AxisListType.X, op=add)
                rz = epip.tile([P, H], f32, tag="rz")
                nc.vector.reciprocal(rz[:], z[:])

                # --- epilogue: combine halves, scale by 1/Z, + b_in ---
                agg = epip.tile([P, C], f32, tag="agg")
                nc.vector.tensor_tensor(out=agg[:], in0=oa[:, 0, :],
                                        in1=oa[:, 1, :], op=add)
                aggb = epip.tile([P, C], bf16, tag="aggb")
                nc.vector.tensor_tensor(
                    out=aggb[:].rearrange("p (h d) -> p h d", d=D),
                    in0=agg[:].rearrange("p (h d) -> p h d", d=D),
                    in1=rz[:, :, None].broadcast_to([P, H, D]), op=mult)

                # --- output projection: out = (agg+b_in) @ W_out^T + b_out ---
                aggT = epip.tile([P, 2, P], bf16, tag="aggT")
                for cc in range(2):
                    pst = psump.tile([P, P], bf16, tag="pst")
                    nc.tensor.transpose(pst[:], aggb[:, cc * P:(cc + 1) * P],
                                        identb[:])
                    nc.scalar.copy(aggT[:, cc, :], pst[:])
                psO = psump.tile([P, C], f32, tag="psO")
                for cc in range(2):
                    nc.tensor.matmul(psO[:], lhsT=aggT[:, cc, :],
                                     rhs=woT[:, cc, :],
                                     start=(cc == 0), stop=(cc == 1))
                out_sb = epip.tile([P, C], f32, tag="outsb")
                nc.vector.tensor_tensor(out=out_sb[:], in0=psO[:],
                                        in1=boutmat[:], op=add)
                nc.sync.dma_start(d_out[qb], out_sb[:])

    nc.compile()
    return nc


def idxw_slice(idxw_tile, qb, k, nic):
    """Column slice of the wrapped idx tile for gather chunk k (nic idxs)."""
    ncols = nic // 16
    return idxw_tile[:, qb, k * ncols:(k + 1) * ncols]


def _wrap_idx(lst):
    """int16 list -> [128, len/16] wrapped (pos i -> [i%16, i//16]) and
    replicated across the 8 groups of 16 partitions."""
    n = lst.shape[0]
    w = np.empty((P, n // 16), np.int16)
    blk = lst.reshape(n // 16, 16).T  # [16, n/16]
    for g in range(8):
        w[g * 16:(g + 1) * 16, :] = blk
    return w


def _bf16(x):
    import ml_dtypes
    return np.ascontiguousarray(np.asarray(x, np.float32)
                                .astype(ml_dtypes.bfloat16))


def make_core_inputs(pairwise_g, coset_functions, nbhd_idx,
                     W_q, b_q, W_k, W_l, u, v, W_in, b_in, W_out, b_out):
    pairwise_g = np.asarray(pairwise_g)
    coset_functions = np.asarray(coset_functions)
    nbhd_idx = np.asarray(nbhd_idx)
    W_q = np.asarray(W_q, np.float32)
    W_k = np.asarray(W_k, np.float32)
    W_l = np.asarray(W_l, np.float32)
    W_in = np.asarray(W_in, np.float32)
    W_out = np.asarray(W_out, np.float32)
    b_q = np.asarray(b_q, np.float32)
    b_in = np.asarray(b_in, np.float32)
    b_out = np.asarray(b_out, np.float32)
    v = np.asarray(v, np.float32)

    wqT = _bf16(W_q.T.reshape(2, P, C))
    # combined K || V rows (V = W_in projection; W_out applied on device)
    wkvT = _bf16(np.concatenate([W_k.T, W_in.T], axis=1).reshape(2, P, 2 * C))
    woT = _bf16(W_out.T.reshape(2, P, C))
    wlBD_full = np.zeros((C, H * POS), np.float32)
    for h in range(H):
        wlBD_full[h * D:(h + 1) * D, h * POS:(h + 1) * POS] = \
            W_l[h * D:(h + 1) * D, :]
    wlBD = _bf16(wlBD_full.reshape(2, P, H * POS))
    bqv = np.ascontiguousarray(
        (b_q + v.reshape(C)).reshape(2, P, 1).astype(np.float32))
    b_out2 = b_out + b_in @ W_out.T  # (agg+b_in)Wo^T+b_out folded
    brows = _bf16(np.stack([b_q, b_in, b_out2])[None, :, :])

    in_maps = []
    for core in range(NCORES):
        b = core // 4
        qs = (core % 4) * NQ
        cosetT = _bf16(coset_functions[b].T.reshape(2, P, N))
        cosetQT = _bf16(coset_functions[b, qs:qs + NQ].T.reshape(2, P, NQ))
        idx = nbhd_idx[b, qs:qs + NQ].astype(np.int64)  # [NQ, M]

        idxw = np.empty((P, QB, M * P // 16), np.int16)
        for qb in range(QB):
            blk = idx[qb * P:(qb + 1) * P]  # [P(n), M]
            # m-major list: pos i = m*128 + n
            lst = blk.T.reshape(M * P)  # [m, n] flattened
            idxw[:, qb, :] = _wrap_idx(lst.astype(np.int16))

        # neighborhood-gathered pairwise_g rows (host-side input sharding):
        # pgrows[n_sub, qb, m, :] = pairwise_g[b, qs+qb*128+n_sub, idx, :]
        pgr = pairwise_g[b, qs + np.arange(NQ)[:, None], idx]  # [NQ, M, POS]
        pgrows = _bf16(pgr.reshape(QB, P, M, POS).transpose(1, 0, 2, 3))

        in_maps.append({
            "cosetT": cosetT, "cosetQT": cosetQT,
            "wqT": wqT, "wkvT": wkvT, "woT": woT,
            "wlBD": wlBD, "bqv": bqv, "brows": brows,
            "idxw": idxw, "pgrows": pgrows,
        })
    return in_maps


def assemble_output(results):
    out = np.empty((B, N, C), np.float32)
    for core in range(NCORES):
        b = core // 4
        qs = (core % 4) * NQ
        o = results[core]["out"]  # [QB, P, C]
        out[b, qs:qs + NQ] = o.reshape(NQ, C).astype(np.float32)
    return out


def kernel(pairwise_g, coset_functions, mask, nbhd_idx,
           W_q, b_q, W_k, b_k, W_l, b_l, u, v,
           W_in, b_in, W_out, b_out, **_unused):
    from concourse.bass_utils import run_bass_kernel_spmd

    if "nc" not in _compiled:
        _compiled["nc"] = build_bass()
    nc = _compiled["nc"]

    in_maps = make_core_inputs(pairwise_g, coset_functions, nbhd_idx,
                               W_q, b_q, W_k, W_l, u, v, W_in, b_in,
                               W_out, b_out)
    res = run_bass_kernel_spmd(nc, in_maps, core_ids=list(range(NCORES)))
    return assemble_output(res.results)
